# revision 16
# baseline (speedup 1.0000x reference)
"""Trainium2 Bass kernel for Dynamic ReLU-B (nn_Dynamic_Relu_B_70291434766473).

Reference computation (per sample n, channel c, pixel p):
    pooled[n,c] = mean_p x[n,c,p]
    h = relu(pooled @ fc1_w.T + fc1_b)                       # [N, 32]
    delta = 2*sigmoid(einsum('koh,nh->kno', fc2_w, h) + fc2_b) - 1
    alpha = delta[..., 0::2]; beta = delta[..., 1::2]        # [K, N, C]
    a = [1,0][k] + 1.0*alpha ; b = [1,0][k] + 0.5*beta
    out = max_k (x * a[k] + b[k])

Strategy: pure data parallel over batch N=32 across 8 NeuronCores (4
samples/core), with all bulk tensors in bf16 (the correctness gate is
rel_err < 2e-2; bf16 x/out keeps worst-case error ~1e-2):

  - x ships to the device as bf16 [4, 256, 3136]; out returns as bf16.
    Per-core HBM traffic is 2 x 6.4 MB = 12.9 MB, ~36-38 us at the
    ~358 GB/s per-NeuronCore HBM limit -- the roofline for this kernel.
  - ALL x loads and out stores ride the single SP HWDGE ring (nc.sync):
    loads are issued first (FIFO), stores queue behind and drain as
    compute completes, so the ring never idles and the ACT engine never
    pays the ~650ns/DMA trigger cost.  Small constants go via SWDGE
    (gpsimd queues).
  - pooling: activation(Copy, accum_out) on ACT / tensor_scalar with
    accum_out on DVE (bf16 2x) / on Pool(GpSimd), split per a static
    assignment table tuned from the trace.
  - per-sample MLP (fp32): fc1 = 2 PSUM-accumulated matmuls on the
    pooled sums (1/HW folded into w1t host-side); fc2 = 2 matmuls with
    [33, 1024] weights moving (bias via ones-row); 2*sigmoid(z)-1
    evaluated as tanh(z/2).  The per-channel a/b params come out of the
    PE transposes directly: beta chunks transpose against a 0.5-scaled
    identity, and the k=0 "+1" init rides an extra ones-row matmul into
    the same PSUM tile, so no DVE prep ops are needed.
  - apply per tile (bf16): y1 = x*a1+b1 (ACT activation or DVE
    tensor_scalar 4x per table), y0 = x*a0+b0 (DVE tensor_scalar 4x),
    out = max(y0, y1) (DVE tensor_tensor 2x), store.
"""

import numpy as np
import ml_dtypes

N, C, H, W = 32, 256, 56, 56
HW = H * W
HID = C // 8  # 32
NCORES = 8
NPC = N // NCORES  # samples per core

# --- engine assignment tables (tuned from traces) -------------------------
A, V, G = "act", "dve", "gps"
POOL_ENG = {
    (0, 0): A, (0, 1): V,
    (1, 0): A, (1, 1): V,
    (2, 0): A, (2, 1): A,
    (3, 0): A, (3, 1): A,
}
Y1_ENG = {
    (0, 0): V, (0, 1): V,
    (1, 0): G, (1, 1): G,
    (2, 0): G, (2, 1): G,
    (3, 0): A, (3, 1): V,
}

_CACHE = {}


def _build_program():
    """Build (and cache) the compiled Bass program for one core."""
    if "nc" in _CACHE:
        return _CACHE["nc"]

    import concourse.bacc as bacc
    import concourse.mybir as mybir
    import concourse.tile as tile

    f32 = mybir.dt.float32
    bf16 = mybir.dt.bfloat16
    AF = mybir.ActivationFunctionType
    ALU = mybir.AluOpType

    nc = bacc.Bacc(
        "TRN2",
        target_bir_lowering=False,
        debug=False,
        enable_asserts=True,
        num_devices=NCORES,
    )

    xs = nc.dram_tensor("xs", [NPC, C, HW], bf16, kind="ExternalInput").ap()
    w1t = nc.dram_tensor("w1t", [C, HID], f32, kind="ExternalInput").ap()
    fc1b = nc.dram_tensor("fc1b", [HID, 1], f32, kind="ExternalInput").ap()
    w2r = nc.dram_tensor("w2r", [HID + 1, 8 * 128], f32, kind="ExternalInput").ap()
    ident = nc.dram_tensor("ident", [1, 130], f32, kind="ExternalInput").ap()
    out = nc.dram_tensor("out", [NPC, C, HW], bf16, kind="ExternalOutput").ap()

    with tile.TileContext(nc) as tc:
        with (
            tc.tile_pool(name="const", bufs=1) as cpool,
            tc.tile_pool(name="x", bufs=2 * NPC) as xpool,
            tc.tile_pool(name="scr", bufs=4) as spool,
            tc.tile_pool(name="y1", bufs=4) as ypool,
            tc.tile_pool(name="o", bufs=4) as opool,
            tc.tile_pool(name="small", bufs=1) as smpool,
            tc.tile_pool(name="ps", bufs=2, space="PSUM") as pspool,
        ):
            # --- all x loads on the SP HWDGE ring, sample-major; issued
            # FIRST so the ring starts immediately -----------------------
            xt = {}
            for n in range(NPC):
                for ch in range(2):
                    t = xpool.tile([128, HW], bf16, tag="x")
                    nc.sync.dma_start(t[:], xs[n, ch * 128:(ch + 1) * 128, :])
                    xt[(n, ch)] = t

            # --- constants (SWDGE queues; don't block the HWDGE ring) ---
            w1t_t = []
            for ch in range(2):
                t = cpool.tile([128, HID], f32, tag=f"w1t{ch}")
                nc.gpsimd.dma_start(t[:], w1t[ch * 128:(ch + 1) * 128, :])
                w1t_t.append(t)
            fc1b_t = cpool.tile([HID, 1], f32, tag="fc1b")
            nc.gpsimd.dma_start(fc1b_t[:], fc1b[:])
            w2r_t = cpool.tile([HID + 1, 8 * 128], f32, tag="w2r")
            nc.gpsimd.dma_start(w2r_t[:], w2r[:])
            # cols: [0]=1.0 (ident), [1]=0.5 (scaled ident), [2:130]=ones
            id_t = cpool.tile([1, 130], f32, tag="ident")
            nc.gpsimd.dma_start(id_t[:], ident[:])

            # persistent h matrix [33, NPC]: row HID = ones (fc2 bias trick)
            ht = smpool.tile([HID + 1, NPC], f32, tag="ht")
            nc.scalar.activation(
                ht[HID:HID + 1, :], id_t[0:1, 0:NPC], AF.Copy, bias=1.0,
                scale=0.0,
            )

            pl, tts = {}, {}

            def pool_sample(n):
                # free-dim sum -> [128, 1] accum per (n, ch); the full-size
                # dump target rotates through spool.  1/HW is folded into
                # the fc1 weights host-side.
                for ch in range(2):
                    scr = spool.tile([128, HW], bf16, tag="scr")
                    p = smpool.tile([128, 1], f32, tag=f"pl{n}{ch}")
                    eng = POOL_ENG[(n, ch)]
                    if eng == A:
                        nc.scalar.activation(
                            scr[:], xt[(n, ch)][:], AF.Copy, accum_out=p[:],
                        )
                    else:
                        nc.vector.tensor_scalar(
                            scr[:], xt[(n, ch)][:], 1.0, None,
                            ALU.mult, ALU.add, accum_out=p[:],
                        )
                    pl[(n, ch)] = p

            def mlp_sample(n):
                # fc1: ph = (fc1_w/HW) @ xsum (2-term PSUM accumulation)
                ph = pspool.tile([HID, 1], f32, tag="ph")
                for ch in range(2):
                    nc.tensor.matmul(
                        ph[:], w1t_t[ch][:], pl[(n, ch)][:],
                        start=(ch == 0), stop=(ch == 1),
                    )
                nc.scalar.activation(
                    ht[0:HID, n:n + 1], ph[:], AF.Relu, bias=fc1b_t[:],
                    scale=1.0,
                )
                # fc2 directly transposed: column j of tp is chunk j of z,
                # via stationary w2r[:, j*128:(j+1)*128] ([33, 128]) and
                # moving ht[:, n] ([33, 1]).  No separate PE transposes.
                tp = pspool.tile([128, 8], f32, tag="tp")
                for j in range(8):
                    nc.tensor.matmul(
                        tp[:, j:j + 1], w2r_t[:, j * 128:(j + 1) * 128],
                        ht[:, n:n + 1], start=True, stop=True,
                        skip_group_check=True,
                    )
                # t = tanh(z/2) = 2*sigmoid(z) - 1, straight off PSUM
                tt = smpool.tile([128, 8], f32, tag=f"tt{n}")
                nc.scalar.activation(tt[:], tp[:], AF.Tanh, bias=0.0, scale=0.5)
                # col j = k*4 + isbeta*2 + ch; derived params in ab:
                #   ab[0:2] a0 = 1 + t ; ab[2:4] b0 = 1 + 0.5 t
                #   ab[6:8] b1 = 0.5 t  (a1 = tt[4:6] used raw)
                ab = smpool.tile([128, 8], f32, tag=f"ab{n}")
                nc.vector.tensor_scalar_add(ab[:, 0:2], tt[:, 0:2], 1.0)
                nc.vector.tensor_scalar(
                    ab[:, 2:4], tt[:, 2:4], 0.5, 1.0, ALU.mult, ALU.add
                )
                nc.vector.tensor_scalar_mul(ab[:, 6:8], tt[:, 6:8], 0.5)
                tts[n] = (tt, ab)

            def apply_sample(n):
                # y1 first (per-table engine), then y0 + max on DVE, then
                # the store on the SP ring (behind all loads, FIFO).
                tt, ab = tts[n]
                y1s = {}
                for ch in range(2):
                    y1 = ypool.tile([128, HW], bf16, tag="y1")
                    eng = Y1_ENG[(n, ch)]
                    if eng == A:
                        nc.scalar.activation(
                            y1[:], xt[(n, ch)][:], AF.Identity,
                            bias=ab[:, 6 + ch:7 + ch],
                            scale=tt[:, 4 + ch:5 + ch],
                        )
                    elif eng == G:
                        nc.gpsimd.tensor_scalar(
                            y1[:], xt[(n, ch)][:],
                            tt[:, 4 + ch:5 + ch], ab[:, 6 + ch:7 + ch],
                            ALU.mult, ALU.add,
                        )
                    else:
                        nc.vector.tensor_scalar(
                            y1[:], xt[(n, ch)][:],
                            tt[:, 4 + ch:5 + ch], ab[:, 6 + ch:7 + ch],
                            ALU.mult, ALU.add,
                        )
                    y1s[ch] = y1
                os_ = {}
                for ch in range(2):
                    o = opool.tile([128, HW], bf16, tag="o")
                    nc.vector.tensor_scalar(
                        o[:], xt[(n, ch)][:],
                        ab[:, 0 + ch:1 + ch], ab[:, 2 + ch:3 + ch],
                        ALU.mult, ALU.add,
                    )
                    os_[ch] = o
                for ch in range(2):
                    o = os_[ch]
                    nc.vector.tensor_max(o[:], o[:], y1s[ch][:])
                    nc.sync.dma_start(out[n, ch * 128:(ch + 1) * 128, :], o[:])

            # pools lead (they pace on DMA arrival); each sample's MLP as
            # soon as it is pooled; applies stream behind.
            pool_sample(0)
            mlp_sample(0)
            pool_sample(1)
            mlp_sample(1)
            pool_sample(2)
            apply_sample(0)
            mlp_sample(2)
            pool_sample(3)
            apply_sample(1)
            mlp_sample(3)
            apply_sample(2)
            apply_sample(3)

    nc.compile()
    _CACHE["nc"] = nc
    return nc


def make_inputs(x, fc1_w, fc1_b, fc2_w, fc2_b):
    """Host-side prep: shard x, rearrange weights into device layouts."""
    x = np.ascontiguousarray(x, dtype=np.float32).reshape(N, C, HW)
    xbf = x.astype(ml_dtypes.bfloat16)
    # fc1: transpose + fold the 1/HW pooling normalizer into the weights
    w1t = np.ascontiguousarray(fc1_w.T.astype(np.float32) / np.float32(HW))
    fc1b = np.ascontiguousarray(fc1_b.astype(np.float32).reshape(HID, 1))
    # fc2 as the *moving* matmul operand: [HID+1, 1024] with col o=j*128+c,
    # j = k*4 + isbeta*2 + ch; row HID carries fc2_b (ones-row trick)
    w2r = np.zeros((HID + 1, 8 * 128), np.float32)
    for k in range(2):
        for isbeta in range(2):
            wab = fc2_w[k, isbeta::2, :].astype(np.float32)  # [256, 32]
            bab = fc2_b[k, isbeta::2].astype(np.float32)     # [256]
            for ch in range(2):
                j = k * 4 + isbeta * 2 + ch
                sl = slice(j * 128, (j + 1) * 128)
                w2r[:HID, sl] = wab[128 * ch:128 * (ch + 1), :].T
                w2r[HID, sl] = bab[128 * ch:128 * (ch + 1)]
    # cols: [0]=1.0 (ident), [1]=0.5 (scaled ident), [2:130]=ones
    ident = np.ones((1, 130), np.float32)
    ident[0, 1] = 0.5
    in_maps = []
    for i in range(NCORES):
        in_maps.append({
            "xs": np.ascontiguousarray(xbf[NPC * i:NPC * (i + 1)]),
            "w1t": w1t,
            "fc1b": fc1b,
            "w2r": w2r,
            "ident": ident,
        })
    return in_maps


def kernel(x, fc1_w, fc1_b, fc2_w, fc2_b):
    from concourse.bass_utils import run_bass_kernel_spmd

    nc = _build_program()
    in_maps = make_inputs(x, fc1_w, fc1_b, fc2_w, fc2_b)
    res = run_bass_kernel_spmd(nc, in_maps, core_ids=list(range(NCORES)))
    shards = [np.asarray(res.results[i]["out"]) for i in range(NCORES)]
    full = np.concatenate(shards, axis=0).astype(np.float32)
    return full.reshape(N, C, H, W)


if __name__ == "__main__":
    rng = np.random.default_rng(0)
    x = rng.standard_normal((N, C, H, W), dtype=np.float32)
    fc1_w = rng.standard_normal((HID, C), dtype=np.float32) * 0.06
    fc1_b = rng.standard_normal((HID,), dtype=np.float32) * 0.06
    fc2_w = rng.standard_normal((2, 2 * C, HID), dtype=np.float32) * 0.17
    fc2_b = rng.standard_normal((2, 2 * C), dtype=np.float32) * 0.17
    out = kernel(x, fc1_w, fc1_b, fc2_w, fc2_b)
    print(out.shape, out.dtype)


# revision 17
# speedup vs baseline: 1.0677x; 1.0677x over previous
"""Trainium2 Bass kernel for Dynamic ReLU-B (nn_Dynamic_Relu_B_70291434766473).

Reference computation (per sample n, channel c, pixel p):
    pooled[n,c] = mean_p x[n,c,p]
    h = relu(pooled @ fc1_w.T + fc1_b)                       # [N, 32]
    delta = 2*sigmoid(einsum('koh,nh->kno', fc2_w, h) + fc2_b) - 1
    alpha = delta[..., 0::2]; beta = delta[..., 1::2]        # [K, N, C]
    a = [1,0][k] + 1.0*alpha ; b = [1,0][k] + 0.5*beta
    out = max_k (x * a[k] + b[k])

Strategy: pure data parallel over batch N=32 across 8 NeuronCores (4
samples/core), with all bulk tensors in bf16 (the correctness gate is
rel_err < 2e-2; bf16 keeps worst-case error ~1e-2):

  - x ships as bf16 [4, 256, 3136]; out returns bf16.  Per-core HBM
    traffic 2 x 6.4 MB = 12.9 MB, ~38-40 us at the ~358 GB/s
    per-NeuronCore HBM limit -- the roofline for this kernel.
  - ALL x loads and out stores ride the single SP HWDGE ring
    (nc.sync): loads issue first (FIFO), stores drain behind them, so
    no other engine pays DMA trigger costs.  Constants go via SWDGE.
  - pooling runs in pixel-halves so the two halves of a tile pool
    concurrently on ACT (activation Copy accum, 1.6us/half) and DVE
    (tensor_scalar accum, 1.8us/half) per a static table; fc1
    accumulates the 4 partial sums per sample in PSUM.
  - per-sample MLP (fp32): fc1 = PSUM-accumulated matmuls; fc2 emits
    the *transposed* result directly (stationary w2r[:, j*128:...]
    [33,128] x moving h [33,1] -> column j), so tanh runs on [128, 8]
    straight off PSUM -- no PE transposes, no [1,1024] activation.
    The derived a/b params are 3 tiny GpSimd tensor_scalar ops.
  - apply per tile (bf16): y1 = x*a1+b1 (DVE tensor_scalar 4x, or ACT
    activation for late samples), y0 = x*a0+b0 (DVE 4x), out =
    max(y0, y1) (DVE tensor_tensor 2x), store.  GpSimd does NO bulk
    streaming: its SBUF port is shared with DVE and measured contention
    knocks DVE tensor_scalar from 4x to 1x.
"""

import numpy as np
import ml_dtypes

N, C, H, W = 32, 256, 56, 56
HW = H * W
HH = HW // 2
HID = C // 8  # 32
NCORES = 8
NPC = N // NCORES  # samples per core

# --- engine assignment tables (tuned from traces) -------------------------
A, V, G = "act", "dve", "gps"
# pool halves: (n, ch, h) -> engine; default ACT, these go to DVE
POOL_DVE_HALVES = {(0, 1, 1), (1, 1, 1)}
# y1 tiles: default DVE, these go to ACT
Y1_ACT = {(2, 0), (2, 1)}
SPLIT_LOAD = (3,)  # samples whose x tiles load in pixel-halves

_CACHE = {}


def _build_program():
    """Build (and cache) the compiled Bass program for one core."""
    if "nc" in _CACHE:
        return _CACHE["nc"]

    import concourse.bacc as bacc
    import concourse.mybir as mybir
    import concourse.tile as tile

    f32 = mybir.dt.float32
    bf16 = mybir.dt.bfloat16
    AF = mybir.ActivationFunctionType
    ALU = mybir.AluOpType

    nc = bacc.Bacc(
        "TRN2",
        target_bir_lowering=False,
        debug=False,
        enable_asserts=False,
        num_devices=NCORES,
    )

    xs = nc.dram_tensor("xs", [NPC, C, HW], bf16, kind="ExternalInput").ap()
    w1t = nc.dram_tensor("w1t", [C, HID], f32, kind="ExternalInput").ap()
    fc1b = nc.dram_tensor("fc1b", [HID, 1], f32, kind="ExternalInput").ap()
    w2r = nc.dram_tensor("w2r", [HID + 1, 8 * 128], f32, kind="ExternalInput").ap()
    ident = nc.dram_tensor("ident", [1, 130], f32, kind="ExternalInput").ap()
    out = nc.dram_tensor("out", [NPC, C, HW], bf16, kind="ExternalOutput").ap()

    with tile.TileContext(nc) as tc:
        with (
            tc.tile_pool(name="const", bufs=1) as cpool,
            tc.tile_pool(name="x", bufs=2 * NPC) as xpool,
            tc.tile_pool(name="scr", bufs=4) as spool,
            tc.tile_pool(name="y1", bufs=4) as ypool,
            tc.tile_pool(name="o", bufs=4) as opool,
            tc.tile_pool(name="small", bufs=1) as smpool,
            tc.tile_pool(name="ps", bufs=2, space="PSUM") as pspool,
        ):
            # --- all x loads on the SP HWDGE ring, sample-major; issued
            # FIRST so the ring starts immediately -----------------------
            xt = {}
            for n in range(NPC):
                for ch in range(2):
                    t = xpool.tile([128, HW], bf16, tag="x")
                    if n in SPLIT_LOAD:
                        for h in range(2):
                            nc.sync.dma_start(
                                t[:, h * HH:(h + 1) * HH],
                                xs[n, ch * 128:(ch + 1) * 128,
                                   h * HH:(h + 1) * HH],
                            )
                    else:
                        nc.sync.dma_start(
                            t[:], xs[n, ch * 128:(ch + 1) * 128, :]
                        )
                    xt[(n, ch)] = t

            # --- constants (SWDGE queues; don't block the HWDGE ring) ---
            w1t_t = []
            for ch in range(2):
                t = cpool.tile([128, HID], f32, tag=f"w1t{ch}")
                nc.gpsimd.dma_start(t[:], w1t[ch * 128:(ch + 1) * 128, :])
                w1t_t.append(t)
            fc1b_t = cpool.tile([HID, 1], f32, tag="fc1b")
            nc.gpsimd.dma_start(fc1b_t[:], fc1b[:])
            w2r_t = cpool.tile([HID + 1, 8 * 128], f32, tag="w2r")
            nc.gpsimd.dma_start(w2r_t[:], w2r[:])
            # cols: [0]=1.0, [1]=0.5, [2:130]=ones (only col 0 used now)
            id_t = cpool.tile([1, 130], f32, tag="ident")
            nc.gpsimd.dma_start(id_t[:], ident[:])

            # persistent h matrix [33, NPC]: row HID = ones (fc2 bias trick)
            ht = smpool.tile([HID + 1, NPC], f32, tag="ht")
            nc.scalar.activation(
                ht[HID:HID + 1, :], id_t[0:1, 0:NPC], AF.Copy, bias=1.0,
                scale=0.0,
            )

            pl, tts = {}, {}

            def pool_sample(n):
                # halves pool concurrently on ACT/DVE; accum -> [128,1]
                # fp32 per (n, ch, h).  1/HW is folded into w1t host-side.
                for ch in range(2):
                    scr = spool.tile([128, HW], bf16, tag="scr")
                    for h in range(2):
                        sl = slice(h * HH, (h + 1) * HH)
                        p = smpool.tile([128, 1], f32, tag=f"pl{n}{ch}{h}")
                        if (n, ch, h) in POOL_DVE_HALVES:
                            nc.vector.tensor_scalar(
                                scr[:, sl], xt[(n, ch)][:, sl], 1.0, None,
                                ALU.mult, ALU.add, accum_out=p[:],
                            )
                        else:
                            nc.scalar.activation(
                                scr[:, sl], xt[(n, ch)][:, sl], AF.Copy,
                                accum_out=p[:],
                            )
                        pl[(n, ch, h)] = p

            def mlp_sample(n):
                # fc1: ph = (fc1_w/HW) @ xsum (4-term PSUM accumulation)
                ph = pspool.tile([HID, 1], f32, tag="ph")
                terms = [(ch, h) for ch in range(2) for h in range(2)]
                for ti, (ch, h) in enumerate(terms):
                    nc.tensor.matmul(
                        ph[:], w1t_t[ch][:], pl[(n, ch, h)][:],
                        start=(ti == 0), stop=(ti == len(terms) - 1),
                    )
                nc.scalar.activation(
                    ht[0:HID, n:n + 1], ph[:], AF.Relu, bias=fc1b_t[:],
                    scale=1.0,
                )
                # fc2 directly transposed: column j of tp is chunk j of z,
                # via stationary w2r[:, j*128:(j+1)*128] and moving h.
                tp = pspool.tile([128, 8], f32, tag="tp")
                for j in range(8):
                    nc.tensor.matmul(
                        tp[:, j:j + 1], w2r_t[:, j * 128:(j + 1) * 128],
                        ht[:, n:n + 1], start=True, stop=True,
                        skip_group_check=True,
                    )
                # t = tanh(z/2) = 2*sigmoid(z) - 1, straight off PSUM
                tt = smpool.tile([128, 8], f32, tag=f"tt{n}")
                nc.scalar.activation(tt[:], tp[:], AF.Tanh, bias=0.0, scale=0.5)
                # col j = k*4 + isbeta*2 + ch; derived params in ab:
                #   ab[0:2] a0 = 1 + t ; ab[2:4] b0 = 1 + 0.5 t
                #   ab[6:8] b1 = 0.5 t  (a1 = tt[4:6] used raw)
                ab = smpool.tile([128, 8], f32, tag=f"ab{n}")
                nc.gpsimd.tensor_scalar_add(ab[:, 0:2], tt[:, 0:2], 1.0)
                nc.gpsimd.tensor_scalar(
                    ab[:, 2:4], tt[:, 2:4], 0.5, 1.0, ALU.mult, ALU.add
                )
                nc.gpsimd.tensor_scalar_mul(ab[:, 6:8], tt[:, 6:8], 0.5)
                tts[n] = (tt, ab)

            def apply_sample(n):
                # y1 first (ACT for Y1_ACT tiles), then y0 + max on DVE,
                # then the store on the SP ring (behind all loads, FIFO).
                tt, ab = tts[n]
                y1s = {}
                for ch in range(2):
                    y1 = ypool.tile([128, HW], bf16, tag="y1")
                    if (n, ch) in Y1_ACT:
                        nc.scalar.activation(
                            y1[:], xt[(n, ch)][:], AF.Identity,
                            bias=ab[:, 6 + ch:7 + ch],
                            scale=tt[:, 4 + ch:5 + ch],
                        )
                    else:
                        nc.vector.tensor_scalar(
                            y1[:], xt[(n, ch)][:],
                            tt[:, 4 + ch:5 + ch], ab[:, 6 + ch:7 + ch],
                            ALU.mult, ALU.add,
                        )
                    y1s[ch] = y1
                os_ = {}
                for ch in range(2):
                    o = opool.tile([128, HW], bf16, tag="o")
                    nc.vector.tensor_scalar(
                        o[:], xt[(n, ch)][:],
                        ab[:, 0 + ch:1 + ch], ab[:, 2 + ch:3 + ch],
                        ALU.mult, ALU.add,
                    )
                    os_[ch] = o
                for ch in range(2):
                    o = os_[ch]
                    nc.vector.tensor_max(o[:], o[:], y1s[ch][:])
                    nc.sync.dma_start(out[n, ch * 128:(ch + 1) * 128, :], o[:])

            # pools lead (they pace on DMA arrival); each sample's MLP as
            # soon as it is pooled; applies stream behind.
            pool_sample(0)
            mlp_sample(0)
            pool_sample(1)
            mlp_sample(1)
            pool_sample(2)
            apply_sample(0)
            mlp_sample(2)
            pool_sample(3)
            apply_sample(1)
            mlp_sample(3)
            apply_sample(2)
            apply_sample(3)

    nc.compile()
    _CACHE["nc"] = nc
    return nc


def make_inputs(x, fc1_w, fc1_b, fc2_w, fc2_b):
    """Host-side prep: shard x, rearrange weights into device layouts."""
    x = np.ascontiguousarray(x, dtype=np.float32).reshape(N, C, HW)
    xbf = x.astype(ml_dtypes.bfloat16)
    # fc1: transpose + fold the 1/HW pooling normalizer into the weights
    w1t = np.ascontiguousarray(fc1_w.T.astype(np.float32) / np.float32(HW))
    fc1b = np.ascontiguousarray(fc1_b.astype(np.float32).reshape(HID, 1))
    # fc2 as [HID+1, 1024] with col o=j*128+c, j = k*4 + isbeta*2 + ch;
    # row HID carries fc2_b (ones-row trick)
    w2r = np.zeros((HID + 1, 8 * 128), np.float32)
    for k in range(2):
        for isbeta in range(2):
            wab = fc2_w[k, isbeta::2, :].astype(np.float32)  # [256, 32]
            bab = fc2_b[k, isbeta::2].astype(np.float32)     # [256]
            for ch in range(2):
                j = k * 4 + isbeta * 2 + ch
                sl = slice(j * 128, (j + 1) * 128)
                w2r[:HID, sl] = wab[128 * ch:128 * (ch + 1), :].T
                w2r[HID, sl] = bab[128 * ch:128 * (ch + 1)]
    # cols: [0]=1.0 (ident), [1]=0.5 (scaled ident), [2:130]=ones
    ident = np.ones((1, 130), np.float32)
    ident[0, 1] = 0.5
    in_maps = []
    for i in range(NCORES):
        in_maps.append({
            "xs": np.ascontiguousarray(xbf[NPC * i:NPC * (i + 1)]),
            "w1t": w1t,
            "fc1b": fc1b,
            "w2r": w2r,
            "ident": ident,
        })
    return in_maps


def kernel(x, fc1_w, fc1_b, fc2_w, fc2_b):
    from concourse.bass_utils import run_bass_kernel_spmd

    nc = _build_program()
    in_maps = make_inputs(x, fc1_w, fc1_b, fc2_w, fc2_b)
    res = run_bass_kernel_spmd(nc, in_maps, core_ids=list(range(NCORES)))
    shards = [np.asarray(res.results[i]["out"]) for i in range(NCORES)]
    full = np.concatenate(shards, axis=0).astype(np.float32)
    return full.reshape(N, C, H, W)


if __name__ == "__main__":
    rng = np.random.default_rng(0)
    x = rng.standard_normal((N, C, H, W), dtype=np.float32)
    fc1_w = rng.standard_normal((HID, C), dtype=np.float32) * 0.06
    fc1_b = rng.standard_normal((HID,), dtype=np.float32) * 0.06
    fc2_w = rng.standard_normal((2, 2 * C, HID), dtype=np.float32) * 0.17
    fc2_b = rng.standard_normal((2, 2 * C), dtype=np.float32) * 0.17
    out = kernel(x, fc1_w, fc1_b, fc2_w, fc2_b)
    print(out.shape, out.dtype)


# revision 25
# speedup vs baseline: 1.1113x; 1.0408x over previous
"""Trainium2 Bass kernel for Dynamic ReLU-B (nn_Dynamic_Relu_B_70291434766473).

Reference computation (per sample n, channel c, pixel p):
    pooled[n,c] = mean_p x[n,c,p]
    h = relu(pooled @ fc1_w.T + fc1_b)                       # [N, 32]
    delta = 2*sigmoid(einsum('koh,nh->kno', fc2_w, h) + fc2_b) - 1
    alpha = delta[..., 0::2]; beta = delta[..., 1::2]        # [K, N, C]
    a = [1,0][k] + 1.0*alpha ; b = [1,0][k] + 0.5*beta
    out = max_k (x * a[k] + b[k])

Strategy: pure data parallel over batch N=32 across 8 NeuronCores (4
samples/core), with all bulk tensors in bf16 (the correctness gate is
rel_err < 2e-2; bf16 keeps worst-case error ~1e-2):

  - x ships as bf16 [4, 256, 3136]; out returns bf16.  Per-core HBM
    traffic 2 x 6.4 MB = 12.9 MB, ~38-40 us at the ~358 GB/s
    per-NeuronCore HBM limit -- the roofline for this kernel.
  - ALL x loads and out stores ride the single SP HWDGE ring
    (nc.sync): loads issue first (FIFO), stores drain behind them, so
    no other engine pays DMA trigger costs.  Constants go via SWDGE.
  - pooling runs in pixel-halves so the two halves of a tile pool
    concurrently on ACT (activation Copy accum, 1.6us/half) and DVE
    (tensor_scalar accum, 1.8us/half) per a static table; fc1
    accumulates the 4 partial sums per sample in PSUM.
  - per-sample MLP (fp32): fc1 = PSUM-accumulated matmuls; fc2 emits
    the *transposed* result directly (stationary w2r[:, j*128:...]
    [33,128] x moving h [33,1] -> column j), so tanh runs on [128, 8]
    straight off PSUM -- no PE transposes, no [1,1024] activation.
    The derived a/b params are 3 tiny GpSimd tensor_scalar ops.
  - apply per tile (bf16): y1 = x*a1+b1 (DVE tensor_scalar 4x, or ACT
    activation for late samples), y0 = x*a0+b0 (DVE 4x), out =
    max(y0, y1) (DVE tensor_tensor 2x), store.  GpSimd does NO bulk
    streaming: its SBUF port is shared with DVE and measured contention
    knocks DVE tensor_scalar from 4x to 1x.
"""

import numpy as np
import ml_dtypes

N, C, H, W = 32, 256, 56, 56
HW = H * W
HH = HW // 2
HID = C // 8  # 32
NCORES = 8
NPC = N // NCORES  # samples per core

# --- engine assignment tables (tuned from traces) -------------------------
A, V, G = "act", "dve", "gps"
# full-tile pool ops: (n, ch) -> engine
POOL_ENG = {
    (0, 0): A, (0, 1): V,
    (1, 0): A, (1, 1): V,
    (2, 0): A, (2, 1): A,
    (3, 0): A, (3, 1): A,
}
# y1 tiles: (n, ch) -> engine
Y1_ENG = {
    (0, 0): V, (0, 1): V,
    (1, 0): G, (1, 1): G,
    (2, 0): G, (2, 1): G,
    (3, 0): A, (3, 1): A,
}

_CACHE = {}


def _build_program():
    """Build (and cache) the compiled Bass program for one core."""
    if "nc" in _CACHE:
        return _CACHE["nc"]

    import concourse.bacc as bacc
    import concourse.mybir as mybir
    import concourse.tile as tile

    f32 = mybir.dt.float32
    bf16 = mybir.dt.bfloat16
    AF = mybir.ActivationFunctionType
    ALU = mybir.AluOpType

    nc = bacc.Bacc(
        "TRN2",
        target_bir_lowering=False,
        debug=False,
        enable_asserts=False,
        num_devices=NCORES,
    )

    xs = nc.dram_tensor("xs", [NPC, C, HW], bf16, kind="ExternalInput").ap()
    w1t = nc.dram_tensor("w1t", [C, HID], f32, kind="ExternalInput").ap()
    fc1b = nc.dram_tensor("fc1b", [HID, 1], f32, kind="ExternalInput").ap()
    w2r = nc.dram_tensor("w2r", [HID + 1, 8 * 128], bf16, kind="ExternalInput").ap()
    ident = nc.dram_tensor("ident", [1, 130], f32, kind="ExternalInput").ap()
    out = nc.dram_tensor("out", [NPC, C, HW], bf16, kind="ExternalOutput").ap()

    with tile.TileContext(nc) as tc:
        with (
            tc.tile_pool(name="const", bufs=1) as cpool,
            tc.tile_pool(name="x", bufs=2 * NPC) as xpool,
            tc.tile_pool(name="scr", bufs=4) as spool,
            tc.tile_pool(name="y1", bufs=4) as ypool,
            tc.tile_pool(name="o", bufs=4) as opool,
            tc.tile_pool(name="small", bufs=1) as smpool,
            tc.tile_pool(name="ps", bufs=2, space="PSUM") as pspool,
        ):
            # --- constants FIRST (SWDGE): their tiny descriptors must hit
            # the SDMA rings before the bulk x packets, else the ht/fc1
            # consumers stall several us behind queued x transfers --------
            w1t_t = []
            for ch in range(2):
                t = cpool.tile([128, HID], f32, tag=f"w1t{ch}")
                nc.gpsimd.dma_start(t[:], w1t[ch * 128:(ch + 1) * 128, :])
                w1t_t.append(t)
            fc1b_t = cpool.tile([HID, 1], f32, tag="fc1b")
            nc.gpsimd.dma_start(fc1b_t[:], fc1b[:])
            w2r_t = cpool.tile([HID + 1, 8 * 128], bf16, tag="w2r")
            nc.gpsimd.dma_start(w2r_t[:], w2r[:])
            # cols: [0]=1.0, [1]=0.5, [2:130]=ones (only col 0 used now)
            id_t = cpool.tile([1, 130], f32, tag="ident")
            nc.gpsimd.dma_start(id_t[:], ident[:])

            # --- all x loads on the SP HWDGE ring, sample-major ---------
            xt = {}
            for n in range(NPC):
                for ch in range(2):
                    t = xpool.tile([128, HW], bf16, tag="x")
                    nc.sync.dma_start(t[:], xs[n, ch * 128:(ch + 1) * 128, :])
                    xt[(n, ch)] = t

            # persistent h matrix [33, NPC]: row HID = ones (fc2 bias trick)
            ht = smpool.tile([HID + 1, NPC], bf16, tag="ht")
            nc.scalar.activation(
                ht[HID:HID + 1, :], id_t[0:1, 0:NPC], AF.Copy, bias=1.0,
                scale=0.0,
            )

            pl, tts = {}, {}

            def pool_sample(n):
                # full-tile free-dim sum -> accum [128,1] fp32 per (n, ch).
                # 1/HW is folded into w1t host-side.
                for ch in range(2):
                    scr = spool.tile([128, HW], bf16, tag="scr")
                    p = smpool.tile([128, 1], f32, tag=f"pl{n}{ch}")
                    if POOL_ENG[(n, ch)] == V:
                        nc.vector.tensor_scalar(
                            scr[:], xt[(n, ch)][:], 1.0, None,
                            ALU.mult, ALU.add, accum_out=p[:],
                        )
                    else:
                        nc.scalar.activation(
                            scr[:], xt[(n, ch)][:], AF.Copy, accum_out=p[:],
                        )
                    pl[(n, ch)] = p

            def mlp_sample(n):
                # fc1: ph = (fc1_w/HW) @ xsum (2-term PSUM accumulation)
                ph = pspool.tile([HID, 1], f32, tag="ph")
                for ch in range(2):
                    nc.tensor.matmul(
                        ph[:], w1t_t[ch][:], pl[(n, ch)][:],
                        start=(ch == 0), stop=(ch == 1),
                    )
                nc.scalar.activation(
                    ht[0:HID, n:n + 1], ph[:], AF.Relu, bias=fc1b_t[:],
                    scale=1.0,
                )
                # fc2 directly transposed: column j of tp is chunk j of z,
                # via stationary w2r[:, j*128:(j+1)*128] and moving h.
                tp = pspool.tile([128, 8], f32, tag="tp")
                for j in range(8):
                    nc.tensor.matmul(
                        tp[:, j:j + 1], w2r_t[:, j * 128:(j + 1) * 128],
                        ht[:, n:n + 1], start=True, stop=True,
                        skip_group_check=True,
                    )
                # t = tanh(z/2) = 2*sigmoid(z) - 1, straight off PSUM
                tt = smpool.tile([128, 8], f32, tag=f"tt{n}")
                nc.scalar.activation(tt[:], tp[:], AF.Tanh, bias=0.0, scale=0.5)
                # col j = k*4 + isbeta*2 + ch; derived params in ab:
                #   ab[0:2] a0 = 1 + t ; ab[2:4] b0 = 1 + 0.5 t
                #   ab[6:8] b1 = 0.5 t  (a1 = tt[4:6] used raw)
                ab = smpool.tile([128, 8], f32, tag=f"ab{n}")
                nc.gpsimd.tensor_scalar_add(ab[:, 0:2], tt[:, 0:2], 1.0)
                nc.gpsimd.tensor_scalar(
                    ab[:, 2:4], tt[:, 2:4], 0.5, 1.0, ALU.mult, ALU.add
                )
                nc.gpsimd.tensor_scalar_mul(ab[:, 6:8], tt[:, 6:8], 0.5)
                tts[n] = (tt, ab)

            def apply_sample(n):
                # y1 first (per-table engine), then y0 + max on DVE; both
                # channel maxes land in ONE per-sample o tile so the store
                # is a single DMA (fewer sems -> shorter epilogue).
                tt, ab = tts[n]
                y1s = {}
                for ch in range(2):
                    y1 = ypool.tile([128, HW], bf16, tag="y1")
                    eng = Y1_ENG[(n, ch)]
                    if eng == A:
                        nc.scalar.activation(
                            y1[:], xt[(n, ch)][:], AF.Identity,
                            bias=ab[:, 6 + ch:7 + ch],
                            scale=tt[:, 4 + ch:5 + ch],
                        )
                    elif eng == G:
                        nc.gpsimd.tensor_scalar(
                            y1[:], xt[(n, ch)][:],
                            tt[:, 4 + ch:5 + ch], ab[:, 6 + ch:7 + ch],
                            ALU.mult, ALU.add,
                        )
                    else:
                        nc.vector.tensor_scalar(
                            y1[:], xt[(n, ch)][:],
                            tt[:, 4 + ch:5 + ch], ab[:, 6 + ch:7 + ch],
                            ALU.mult, ALU.add,
                        )
                    y1s[ch] = y1
                os_ = {}
                for ch in range(2):
                    o = opool.tile([128, HW], bf16, tag="o")
                    nc.vector.tensor_scalar(
                        o[:], xt[(n, ch)][:],
                        ab[:, 0 + ch:1 + ch], ab[:, 2 + ch:3 + ch],
                        ALU.mult, ALU.add,
                    )
                    os_[ch] = o
                for ch in range(2):
                    o = os_[ch]
                    nc.vector.tensor_max(o[:], o[:], y1s[ch][:])
                    nc.sync.dma_start(out[n, ch * 128:(ch + 1) * 128, :], o[:])

            # pools lead (they pace on DMA arrival); each sample's MLP as
            # soon as it is pooled; applies stream behind.
            pool_sample(0)
            mlp_sample(0)
            pool_sample(1)
            mlp_sample(1)
            pool_sample(2)
            apply_sample(0)
            mlp_sample(2)
            pool_sample(3)
            apply_sample(1)
            mlp_sample(3)
            apply_sample(2)
            apply_sample(3)

    nc.compile()
    _CACHE["nc"] = nc
    return nc


def make_inputs(x, fc1_w, fc1_b, fc2_w, fc2_b):
    """Host-side prep: shard x, rearrange weights into device layouts."""
    x = np.ascontiguousarray(x, dtype=np.float32).reshape(N, C, HW)
    xbf = x.astype(ml_dtypes.bfloat16)
    # fc1: transpose + fold the 1/HW pooling normalizer into the weights
    w1t = np.ascontiguousarray(fc1_w.T.astype(np.float32) / np.float32(HW))
    fc1b = np.ascontiguousarray(fc1_b.astype(np.float32).reshape(HID, 1))
    # fc2 as [HID+1, 1024] with col o=j*128+c, j = k*4 + isbeta*2 + ch;
    # row HID carries fc2_b (ones-row trick)
    w2r = np.zeros((HID + 1, 8 * 128), np.float32)
    for k in range(2):
        for isbeta in range(2):
            wab = fc2_w[k, isbeta::2, :].astype(np.float32)  # [256, 32]
            bab = fc2_b[k, isbeta::2].astype(np.float32)     # [256]
            for ch in range(2):
                j = k * 4 + isbeta * 2 + ch
                sl = slice(j * 128, (j + 1) * 128)
                w2r[:HID, sl] = wab[128 * ch:128 * (ch + 1), :].T
                w2r[HID, sl] = bab[128 * ch:128 * (ch + 1)]
    w2r = w2r.astype(ml_dtypes.bfloat16)
    # cols: [0]=1.0 (ident), [1]=0.5 (scaled ident), [2:130]=ones
    ident = np.ones((1, 130), np.float32)
    ident[0, 1] = 0.5
    in_maps = []
    for i in range(NCORES):
        in_maps.append({
            "xs": np.ascontiguousarray(xbf[NPC * i:NPC * (i + 1)]),
            "w1t": w1t,
            "fc1b": fc1b,
            "w2r": w2r,
            "ident": ident,
        })
    return in_maps


def kernel(x, fc1_w, fc1_b, fc2_w, fc2_b):
    from concourse.bass_utils import run_bass_kernel_spmd

    nc = _build_program()
    in_maps = make_inputs(x, fc1_w, fc1_b, fc2_w, fc2_b)
    res = run_bass_kernel_spmd(nc, in_maps, core_ids=list(range(NCORES)))
    shards = [np.asarray(res.results[i]["out"]) for i in range(NCORES)]
    full = np.concatenate(shards, axis=0).astype(np.float32)
    return full.reshape(N, C, H, W)


if __name__ == "__main__":
    rng = np.random.default_rng(0)
    x = rng.standard_normal((N, C, H, W), dtype=np.float32)
    fc1_w = rng.standard_normal((HID, C), dtype=np.float32) * 0.06
    fc1_b = rng.standard_normal((HID,), dtype=np.float32) * 0.06
    fc2_w = rng.standard_normal((2, 2 * C, HID), dtype=np.float32) * 0.17
    fc2_b = rng.standard_normal((2, 2 * C), dtype=np.float32) * 0.17
    out = kernel(x, fc1_w, fc1_b, fc2_w, fc2_b)
    print(out.shape, out.dtype)
